# revision 45
# baseline (speedup 1.0000x reference)
"""NVFP4 BlackwellLinear kernel for 8 Trainium2 NeuronCores.

Strategy (token-parallel + fp8e4m3 DoubleRow matmul):
  - x is sharded along tokens (4096 -> 8 x 512); weights are replicated.
  - Weights are dequantized on host (w_deq = weight_q * weight_scale, exact in
    6 significand bits), prescaled by 2^8 into the fp8e4m3 normal range, RNE
    cast to fp8, and packed [group, p, ktile, j] so each 512-out-column group
    is one contiguous 2 MiB DMA streamed under the matmul.
  - Each core quantizes only its 512 tokens (amax per 16-block -> e4m3 scale
    via hardware cast -> fp4 round via custom DVE ops -> dequantized bf16),
    transposes on-chip with PE transposes, and the PSUM eviction applies a 2^2
    prescale + fp8e4m3 cast into the resident k-major activation tile.
  - Matmul runs in fp8 DoubleRow perf mode: one MM contracts a PAIR of
    k-tiles via 3D [p, 2, f] APs at 2x bf16 throughput.  k-pairs are issued
    HIGH->LOW so the in-order PE queue blocks exactly once at the
    quant/matmul boundary.  The PSUM->SBUF eviction divides out the 2^-10
    prescale and fuses the bias add.  Host transposes/concats out^T slices.

fp4 round-to-nearest is computed as:
  v2   = clamp(x * (2/s), +-12)                        [custom DVE op Q1]
  qh   = (v2 + sign_binade(v2)*0.25) & 0xFFC00000      [trunc to 1 mantissa bit, Q3A]
  q2   = qh*qh >= 16 ? qh : (v2 + 1.5*2^23) - 1.5*2^23 [select + fused magic RNE, Q3B2]
  xdeq = q2 * (s/2)                                    [bf16 tensor_tensor]
which matches the reference grid exactly except at exact ties (measure-zero);
the final fp8 cast of x_deq/w_deq (4-bit significands) adds ~1e-2 relative
error on the output, within the 2e-2 gate.
"""

import os
import numpy as np

TOK = 4096
K = 4096
OUT_F = 16384
N_CORES = 8
TLOC = TOK // N_CORES  # 512 tokens per core
P = 128
BLOCK = 16

# tunables
QS = 1024            # quant compute slice (free elems)
NGRP = 4             # n-tiles (x128 outs) per streamed weight group
MAGIC = 12582912.0   # 1.5 * 2^23
FP8_MIN = 2.0 ** -9
# fp8 matmul prescales: keep operands in fp8e4m3 normal range; the exact
# power-of-2 product is divided back out in the PSUM eviction.
WSCALE = 256.0       # weights * 2^8  -> [0.25, ~30]
ASCALE = 4.0         # activations * 2^2
OSCALE = 1.0 / (WSCALE * ASCALE)

_REGISTERED = {}


def _register_ops():
    """Register the custom DVE ops (idempotent). shas computed dynamically."""
    if _REGISTERED:
        return _REGISTERED
    import concourse.dve_ops as dve_ops
    from concourse.dve_ops import DveOp
    from concourse.dve_spec import (
        Spec, Src0, Src1, C0, C1, C2, Zero, MaxNeg, lower, AluOp, Bin,
        maxx, minn, select, _has_src1,
    )
    from concourse.dve_uop import DveOpSpec

    def ref_q1(in0, in1, s0, s1, imm2):
        a = np.asarray(in0, np.float32)
        b = np.asarray(in1, np.float32).reshape(a.shape)
        return np.clip((a * b).astype(np.float32), np.float32(-s0), np.float32(s0))

    body_q1 = minn(maxx(Src0 * Src1, Zero - C0), C0)
    spec_q1 = Spec(body=body_q1, reference=ref_q1)

    def ref_q2(in0, in1, s0, s1, imm2):
        v = np.asarray(in0, np.float32)
        return ((v + np.float32(s0)).astype(np.float32) - np.float32(s0)).astype(np.float32)

    spec_q2 = Spec(body=(Src0 + C0) - C0, reference=ref_q2)

    def ref_q3a(in0, in1, s0, s1, imm2):
        v2 = np.asarray(in0, np.float32)
        p = (v2.view(np.uint32) & np.uint32(0xFF800000)).view(np.float32)
        bh = (v2 + p * np.float32(imm2)).astype(np.float32)
        return (bh.view(np.uint32) & np.uint32(0xFFC00000)).view(np.float32)

    # trunc-to-1-mantissa-bit without NaN-pattern masks (NaN sign is mangled
    # on the f32 read path): bh & 0xFFC00000 == (bh & -inf) | (bh & 0x00400000)
    p3 = Bin(AluOp.BITWISE_AND, Src0, C0)  # C0 = -inf mask AP (0xFF800000)
    bh3 = Src0 + p3 * C2
    q3a_hi = Bin(AluOp.BITWISE_AND, bh3, C0)
    q3a_lo = Bin(AluOp.BITWISE_AND, bh3, C1)  # C1 = 0x00400000 subnormal mask AP
    spec_q3a = Spec(body=Bin(AluOp.BITWISE_OR, q3a_hi, q3a_lo), reference=ref_q3a)

    def ref_q3b(in0, in1, s0, s1, imm2):
        qh = np.asarray(in0, np.float32)
        m = np.asarray(in1, np.float32)
        return np.where(qh * qh >= np.float32(imm2), qh, m).astype(np.float32)

    spec_q3b = Spec(body=select(Src0 * Src0 >= C2, Src0, Src1), reference=ref_q3b)

    def ref_q3b2(in0, in1, s0, s1, imm2):
        qh = np.asarray(in0, np.float32)
        v = np.asarray(in1, np.float32)
        m = ((v + np.float32(s0)).astype(np.float32)
             - np.float32(s0)).astype(np.float32)
        return np.where(qh * qh >= np.float32(imm2), qh, m).astype(np.float32)

    # fused magic-RNE + select: m computed inline from the pre-trunc value,
    # freeing the two ACT Copy passes from the quant critical path
    spec_q3b2 = Spec(body=select(Src0 * Src0 >= C2, Src0, (Src1 + C0) - C0),
                     reference=ref_q3b2)

    def mk(name, spec):
        shas = {}
        for ver in ("v3", "v4"):
            uops = lower(spec, ver=ver)
            row = dve_ops._CUSTOM_DVE_ROW_BASE + len(dve_ops.OPS)
            dos = DveOpSpec(name=name, opcode=row, uops=uops, rd1_en=_has_src1(spec))
            shas[ver] = dos.sha(ver)
        op = DveOp(name, spec, subdim=False, uops_sha=shas)
        dve_ops.OPS.append(op)
        dve_ops.CUSTOM_DVE_SPECS[name] = spec
        dve_ops._SUB_OPCODE_FOR_NAME[name] = dve_ops._CUSTOM_DVE_ROW_BASE + len(dve_ops.OPS) - 1
        return op

    _REGISTERED["Q1"] = mk("NVFP4_MULCLAMP_ANT", spec_q1)
    _REGISTERED["Q2"] = mk("NVFP4_MAGICRNE_ANT", spec_q2)
    _REGISTERED["Q3A"] = mk("NVFP4_TRUNC1_ANT", spec_q3a)
    _REGISTERED["Q3B"] = mk("NVFP4_COMBINE_ANT", spec_q3b)
    _REGISTERED["Q3B2"] = mk("NVFP4_COMBRNE_ANT", spec_q3b2)
    return _REGISTERED


_NC_CACHE = {}


def build_nc(tloc=TLOC, k=K, out_f=OUT_F, qs=QS, ngrp=NGRP):
    key = (tloc, k, out_f, qs, ngrp)
    if key in _NC_CACHE:
        return _NC_CACHE[key]

    import concourse.bass as bass
    import concourse.mybir as mybir
    import concourse.tile as tile
    from concourse import bacc, masks

    ops = _register_ops()
    dt = mybir.dt

    KT = k // P              # 32 k-tiles
    MT = tloc // P           # 4 local m-tiles
    NT = out_f // P          # 128 n-tiles
    NG = NT // ngrp          # 32 weight groups
    GW = ngrp * P            # 512 out columns per group

    nc = bacc.Bacc("TRN2", target_bir_lowering=False, debug=False,
                   num_devices=N_CORES)

    x_d = nc.dram_tensor("x", [tloc, k], dt.float32, kind="ExternalInput").ap()
    # weights prepacked on host as [g, p, kt, j] so each group is one
    # contiguous 2 MiB block (sequential HBM reads, 16 KiB per partition line)
    wt_d = nc.dram_tensor("wt", [(out_f // (ngrp * P)) * P, k * ngrp],
                          dt.float8e4, kind="ExternalInput").ap()
    b_d = nc.dram_tensor("bias", [P, NT], dt.float32, kind="ExternalInput").ap()
    o_d = nc.dram_tensor("outT", [out_f, tloc], dt.float32, kind="ExternalOutput").ap()

    with tile.TileContext(nc) as tc:
        with (
            tc.tile_pool(name="const", bufs=1) as constp,
            tc.tile_pool(name="xdT", bufs=1) as xdTp,
            tc.tile_pool(name="xin", bufs=16) as xin,
            tc.tile_pool(name="scal", bufs=3) as scal,
            tc.tile_pool(name="v2p", bufs=3) as v2p,
            tc.tile_pool(name="tp", bufs=3) as tp,
            tc.tile_pool(name="q2p", bufs=3) as q2p,
            tc.tile_pool(name="xqp", bufs=6) as xqp,
            tc.tile_pool(name="shp", bufs=3) as shp,
            tc.tile_pool(name="wgp", bufs=3) as wgp,
            tc.tile_pool(name="outp", bufs=3) as outp,
            tc.tile_pool(name="psum", bufs=6, space="PSUM") as psump,
            tc.tile_pool(name="psumT", bufs=2, space="PSUM") as psumTp,
        ):
            # ---- constants ----
            nmask = constp.tile([P, 1], dt.float32, tag="nmask")
            nc.vector._memset_packed(nmask[:], 0xFF800000)
            smask = constp.tile([P, 1], dt.float32, tag="smask")
            nc.vector._memset_packed(smask[:], 0x00400000)
            bias_t = constp.tile([P, NT], dt.float32, tag="bias")
            ident = constp.tile([P, P], dt.bfloat16, tag="ident")
            masks.make_identity(nc, ident[:])

            # ---- resident transposed dequantized activations (fp8, *ASCALE) ----
            xdTa = xdTp.tile([P, KT * tloc], dt.float8e4, tag="xdTa",
                             name="xdTa")
            xdT3 = xdTa[:].rearrange("p (kt t) -> p kt t", t=tloc)

            nq = k // qs             # quant slices per m-tile row
            nblk = qs // BLOCK       # 16-blocks per quant slice
            ntp = qs // P            # transposes per quant slice

            # ---- prefetch all x slices up front (sync ring, q-major order
            # matching consumption) so quant never waits on input DMA ----
            xts = {}
            for q in range(nq):
                for m in range(MT):
                    xsl = xin.tile([P, qs], dt.float32, tag="xsl", name="xsl")
                    nc.sync.dma_start(
                        xsl[:], x_d[m * P:(m + 1) * P, q * qs:(q + 1) * qs])
                    xts[(m, q)] = xsl
            nc.sync.dma_start(bias_t[:], b_d[:, :])

            # ---- prefetch the first weight groups while quant runs ----
            def wload(g):
                wg = wgp.tile([P, KT * GW], dt.float8e4, tag="wg", name="wg")
                nc.sync.dma_start(wg[:], wt_d[g * P:(g + 1) * P, :])
                return wg

            wgs = [wload(0), wload(1), wload(2)]

            def quant_slice(m, q):
                xv = xts[(m, q)][:]
                # scales
                amax = scal.tile([P, nblk], dt.float32, tag="amax")
                nc.vector.tensor_reduce(
                    amax[:], xv.rearrange("p (b s) -> p b s", s=BLOCK),
                    axis=mybir.AxisListType.X, op=mybir.AluOpType.max,
                    apply_absolute_value=True)
                # s8 on DVE: keeps the ACT queue (which gates overlap MMs via
                # the transpose evictions) out of the DVE dependency chain
                s8 = scal.tile([P, nblk], dt.float8e4, tag="s8")
                nc.vector.tensor_scalar(
                    out=s8[:], in0=amax[:], scalar1=1.0 / 6.0, scalar2=None,
                    op0=mybir.AluOpType.mult)
                sh = scal.tile([P, nblk], dt.float32, tag="sh")
                nc.vector.tensor_scalar(
                    out=sh[:], in0=s8[:], scalar1=FP8_MIN, scalar2=0.5,
                    op0=mybir.AluOpType.max, op1=mybir.AluOpType.mult)
                r2 = scal.tile([P, nblk], dt.float32, tag="r2")
                rs = scal.tile([P, nblk], dt.float32, tag="rs")
                nc.vector.reciprocal_approx_accurate(r2[:], sh[:], rs[:])
                # s/2 expanded to bf16 (ACT)
                shx = shp.tile([P, qs], dt.bfloat16, tag="shx")
                nc.scalar.activation(
                    shx[:].rearrange("p (b s) -> p b s", s=BLOCK),
                    sh[:].unsqueeze(2).to_broadcast((P, nblk, BLOCK)),
                    mybir.ActivationFunctionType.Copy, bias=0.0, scale=1.0)
                # v2 = clamp(x * 2/s, +-12)
                v2 = v2p.tile([P, qs], dt.float32, tag="v2")
                nc.vector._custom_dve(
                    ops["Q1"], out=v2[:], in0=xv,
                    in1=r2[:].unsqueeze(2).to_broadcast((P, nblk, BLOCK)),
                    s0=12.0)
                # qh = trunc1(v2 + sign_binade/4)
                qh = tp.tile([P, qs], dt.float32, tag="qh")
                nc.vector._custom_dve(
                    ops["Q3A"], out=qh[:], in0=v2[:],
                    s0=nmask[:, :], s1=smask[:, :], imm2=0.25)
                # q2 = select(qh^2>=16, qh, RNE-to-int(v2)) -> bf16
                # (magic-number RNE fused into the op; no ACT in the chain)
                q2 = q2p.tile([P, qs], dt.bfloat16, tag="q2")
                nc.vector._custom_dve(
                    ops["Q3B2"], out=q2[:], in0=qh[:], in1=v2[:],
                    s0=MAGIC, imm2=16.0)
                # xdeq = q2 * s/2  (bf16 2x mode)
                xq = xqp.tile([P, qs], dt.bfloat16, tag="xq")
                nc.vector.tensor_tensor(
                    out=xq[:], in0=q2[:], in1=shx[:],
                    op=mybir.AluOpType.mult)
                # on-chip transpose into the resident k-major fp8 tile
                # (eviction applies the *ASCALE prescale and the fp8 cast)
                for j in range(ntp):
                    kt = q * ntp + j
                    pst = psumTp.tile([P, P], dt.bfloat16, tag="pst",
                                      name="pst")
                    nc.tensor.transpose(
                        pst[:], xq[:, j * P:(j + 1) * P], ident[:])
                    nc.scalar.activation(
                        xdTa[:, kt * tloc + m * P: kt * tloc + (m + 1) * P],
                        pst[:],
                        mybir.ActivationFunctionType.Copy, bias=0.0,
                        scale=ASCALE)

            def mm_chain(ps, wg, nt):
                # fp8 DoubleRow: one MM contracts a PAIR of k-tiles via 3D APs
                # [p, 2, f]; out += W_k.T @ X_k + W_{k+1}.T @ X_{k+1}
                # k-pairs run HIGH->LOW: the first issued MM needs the last
                # quantized k-tile, so the in-order PE queue blocks exactly
                # once at the quant/matmul boundary instead of stalling the
                # transpose stream with partially-satisfiable chains.
                wg3 = wg[:].rearrange("p (kt j) -> p kt j", j=GW)
                for ki in range(KT - 2, -2, -2):
                    lhsT = wg3[:, ki:ki + 2, nt * P:(nt + 1) * P]
                    rhs = xdT3[:, ki:ki + 2, :]
                    nc.tensor.matmul(
                        ps[:], lhsT, rhs,
                        start=(ki == KT - 2), stop=(ki == 0),
                        perf_mode=mybir.MatmulPerfMode.DoubleRow)

            def evict(ps, n):
                ob = outp.tile([P, tloc], dt.float32, tag="ob", name="ob")
                nc.scalar.activation(
                    ob[:], ps[:], mybir.ActivationFunctionType.Identity,
                    bias=bias_t[:, n:n + 1], scale=OSCALE)
                nc.scalar.dma_start(o_d[n * P:(n + 1) * P, :], ob[:])

            # ---- quant with overlapped group-0 chains: after each quant
            # q-batch, the 4 group-0 output chains consume exactly the
            # freshly-evicted k-tiles (natural k order), so every issued MM's
            # deps are already satisfied and the PE queue never blocks the
            # transpose stream on a partially-satisfiable chain ----
            OV = 6  # overlapped (g, nt) chains; 6 PSUM banks + 2 transpose banks
            wgr3 = [wgs[0][:].rearrange("p (kt j) -> p kt j", j=GW),
                    wgs[1][:].rearrange("p (kt j) -> p kt j", j=GW)]
            ovch = [(divmod(ci, ngrp),
                     psump.tile([P, tloc], dt.float32, tag="ps", name="ps"))
                    for ci in range(OV)]
            kq = KT // nq  # k-tiles completed per quant q-batch
            for q in range(nq):
                for m in range(MT):
                    quant_slice(m, q)
                for (g, nt), ps in ovch:
                    for ki in range(q * kq, (q + 1) * kq, 2):
                        lhsT = wgr3[g][:, ki:ki + 2, nt * P:(nt + 1) * P]
                        rhs = xdT3[:, ki:ki + 2, :]
                        nc.tensor.matmul(
                            ps[:], lhsT, rhs,
                            start=(ki == 0), stop=(ki == KT - 2),
                            perf_mode=mybir.MatmulPerfMode.DoubleRow)
            for (g, nt), ps in ovch:
                evict(ps, g * ngrp + nt)

            # ---- streamed matmul: out^T[n, t] = w_deq @ x_deq^T ----
            for g in range(NG):
                wg = wgs[g] if g < 3 else wload(g)
                for nt in range(ngrp):
                    if g * ngrp + nt < OV:
                        continue
                    ps = psump.tile([P, tloc], dt.float32, tag="ps", name="ps")
                    mm_chain(ps, wg, nt)
                    evict(ps, g * ngrp + nt)

    nc.compile()
    _NC_CACHE[key] = nc
    return nc


def _prep_weights(weight_q, weight_scale, bias):
    """Host prepack: transposed dequantized bf16 weights (shared by all cores).

    Packed as [g, p, kt, j] (g = 512-out group, p = k-within-tile, kt = k-tile,
    j = out-within-group) so each group is one contiguous 4 MiB DMA with
    32 KiB per SBUF partition line, in the [kt, j] column order the matmul
    slices expect.
    """
    import ml_dtypes
    wq = np.asarray(weight_q, np.float32).reshape(OUT_F, K // BLOCK, BLOCK)
    ws = np.asarray(weight_scale, np.float32)[:, :, None]
    wdeq = (wq * ws).reshape(OUT_F, K)  # exact: <=6 significand bits
    wt = (wdeq.T * np.float32(WSCALE)).astype(ml_dtypes.float8_e4m3)  # [K, OUT_F]
    KT, NG, GW = K // P, OUT_F // (NGRP * P), NGRP * P
    wt_p = np.ascontiguousarray(
        wt.reshape(KT, P, NG, GW).transpose(2, 1, 0, 3)).reshape(NG * P, K * NGRP)
    bias_r = np.ascontiguousarray(
        np.asarray(bias, np.float32).reshape(OUT_F // P, P).T)  # [P, NT]
    return wt_p, bias_r


def kernel(x, weight_q, weight_scale, bias):
    from concourse.bass_utils import run_bass_kernel_spmd

    nc = build_nc()
    x2 = np.ascontiguousarray(np.asarray(x, np.float32).reshape(TOK, K))
    wt, bias_r = _prep_weights(weight_q, weight_scale, bias)
    in_maps = [{"x": x2[c * TLOC:(c + 1) * TLOC], "wt": wt, "bias": bias_r}
               for c in range(N_CORES)]
    res = run_bass_kernel_spmd(nc, in_maps, list(range(N_CORES)))
    out = np.empty((TOK, OUT_F), np.float32)
    for c in range(N_CORES):
        out[c * TLOC:(c + 1) * TLOC, :] = res.results[c]["outT"].T
    return out.reshape(1, TOK, OUT_F)


if __name__ == "__main__":
    rng = np.random.default_rng(0)
    x = rng.normal(size=(1, TOK, K)).astype(np.float32)
    wq = rng.normal(size=(OUT_F, K)).astype(np.float32)
    ws = rng.random(size=(OUT_F, K // BLOCK)).astype(np.float32) + 0.1
    b = rng.normal(size=(OUT_F,)).astype(np.float32)
    out = kernel(x, wq, ws, b)
    print(out.shape, out.dtype)


# revision 46
# speedup vs baseline: 1.1833x; 1.1833x over previous
"""NVFP4 BlackwellLinear kernel for 8 Trainium2 NeuronCores.

Strategy (token-parallel + fp8e4m3 DoubleRow matmul):
  - x is sharded along tokens (4096 -> 8 x 512); weights are replicated.
  - Weights are dequantized on host (w_deq = weight_q * weight_scale, exact in
    6 significand bits), prescaled by 2^8 into the fp8e4m3 normal range, RNE
    cast to fp8, and packed [group, p, ktile, j] so each 512-out-column group
    is one contiguous 2 MiB DMA streamed under the matmul.
  - Each core quantizes only its 512 tokens (amax per 16-block -> e4m3 scale
    via hardware cast -> fp4 round via custom DVE ops -> dequantized bf16),
    transposes on-chip with PE transposes, and the PSUM eviction applies a 2^2
    prescale + fp8e4m3 cast into the resident k-major activation tile.
  - Matmul runs in fp8 DoubleRow perf mode: one MM contracts a PAIR of
    k-tiles via 3D [p, 2, f] APs at 2x bf16 throughput.  k-pairs are issued
    HIGH->LOW so the in-order PE queue blocks exactly once at the
    quant/matmul boundary.  The PSUM->SBUF eviction divides out the 2^-10
    prescale and fuses the bias add.  Host transposes/concats out^T slices.

fp4 round-to-nearest is computed as:
  v2   = clamp(x * (2/s), +-12)                        [custom DVE op Q1]
  qh   = (v2 + sign_binade(v2)*0.25) & 0xFFC00000      [trunc to 1 mantissa bit, Q3A]
  q2   = qh*qh >= 16 ? qh : (v2 + 1.5*2^23) - 1.5*2^23 [select + fused magic RNE, Q3B2]
  xdeq = q2 * (s/2)                                    [bf16 tensor_tensor]
which matches the reference grid exactly except at exact ties (measure-zero);
the final fp8 cast of x_deq/w_deq (4-bit significands) adds ~1e-2 relative
error on the output, within the 2e-2 gate.
"""

import os
import numpy as np

TOK = 4096
K = 4096
OUT_F = 16384
N_CORES = 8
TLOC = TOK // N_CORES  # 512 tokens per core
P = 128
BLOCK = 16

# tunables
QS = 1024            # quant compute slice (free elems)
NGRP = 4             # n-tiles (x128 outs) per streamed weight group
MAGIC = 12582912.0   # 1.5 * 2^23
FP8_MIN = 2.0 ** -9
# fp8 matmul prescales: keep operands in fp8e4m3 normal range; the exact
# power-of-2 product is divided back out in the PSUM eviction.
WSCALE = 256.0       # weights * 2^8  -> [0.25, ~30]
ASCALE = 4.0         # activations * 2^2
OSCALE = 1.0 / (WSCALE * ASCALE)

_REGISTERED = {}


def _register_ops():
    """Register the custom DVE ops (idempotent). shas computed dynamically."""
    if _REGISTERED:
        return _REGISTERED
    import concourse.dve_ops as dve_ops
    from concourse.dve_ops import DveOp
    from concourse.dve_spec import (
        Spec, Src0, Src1, C0, C1, C2, Zero, MaxNeg, lower, AluOp, Bin,
        maxx, minn, select, _has_src1,
    )
    from concourse.dve_uop import DveOpSpec

    def ref_q1(in0, in1, s0, s1, imm2):
        a = np.asarray(in0, np.float32)
        b = np.asarray(in1, np.float32).reshape(a.shape)
        return np.clip((a * b).astype(np.float32), np.float32(-s0), np.float32(s0))

    body_q1 = minn(maxx(Src0 * Src1, Zero - C0), C0)
    spec_q1 = Spec(body=body_q1, reference=ref_q1)

    def ref_q2(in0, in1, s0, s1, imm2):
        v = np.asarray(in0, np.float32)
        return ((v + np.float32(s0)).astype(np.float32) - np.float32(s0)).astype(np.float32)

    spec_q2 = Spec(body=(Src0 + C0) - C0, reference=ref_q2)

    def ref_q3a(in0, in1, s0, s1, imm2):
        v2 = np.asarray(in0, np.float32)
        p = (v2.view(np.uint32) & np.uint32(0xFF800000)).view(np.float32)
        bh = (v2 + p * np.float32(imm2)).astype(np.float32)
        return (bh.view(np.uint32) & np.uint32(0xFFC00000)).view(np.float32)

    # trunc-to-1-mantissa-bit without NaN-pattern masks (NaN sign is mangled
    # on the f32 read path): bh & 0xFFC00000 == (bh & -inf) | (bh & 0x00400000)
    p3 = Bin(AluOp.BITWISE_AND, Src0, C0)  # C0 = -inf mask AP (0xFF800000)
    bh3 = Src0 + p3 * C2
    q3a_hi = Bin(AluOp.BITWISE_AND, bh3, C0)
    q3a_lo = Bin(AluOp.BITWISE_AND, bh3, C1)  # C1 = 0x00400000 subnormal mask AP
    spec_q3a = Spec(body=Bin(AluOp.BITWISE_OR, q3a_hi, q3a_lo), reference=ref_q3a)

    def ref_q3b(in0, in1, s0, s1, imm2):
        qh = np.asarray(in0, np.float32)
        m = np.asarray(in1, np.float32)
        return np.where(qh * qh >= np.float32(imm2), qh, m).astype(np.float32)

    spec_q3b = Spec(body=select(Src0 * Src0 >= C2, Src0, Src1), reference=ref_q3b)

    def ref_q3b2(in0, in1, s0, s1, imm2):
        qh = np.asarray(in0, np.float32)
        v = np.asarray(in1, np.float32)
        m = ((v + np.float32(s0)).astype(np.float32)
             - np.float32(s0)).astype(np.float32)
        return np.where(qh * qh >= np.float32(imm2), qh, m).astype(np.float32)

    # fused magic-RNE + select: m computed inline from the pre-trunc value,
    # freeing the two ACT Copy passes from the quant critical path
    spec_q3b2 = Spec(body=select(Src0 * Src0 >= C2, Src0, (Src1 + C0) - C0),
                     reference=ref_q3b2)

    def mk(name, spec):
        shas = {}
        for ver in ("v3", "v4"):
            uops = lower(spec, ver=ver)
            row = dve_ops._CUSTOM_DVE_ROW_BASE + len(dve_ops.OPS)
            dos = DveOpSpec(name=name, opcode=row, uops=uops, rd1_en=_has_src1(spec))
            shas[ver] = dos.sha(ver)
        op = DveOp(name, spec, subdim=False, uops_sha=shas)
        dve_ops.OPS.append(op)
        dve_ops.CUSTOM_DVE_SPECS[name] = spec
        dve_ops._SUB_OPCODE_FOR_NAME[name] = dve_ops._CUSTOM_DVE_ROW_BASE + len(dve_ops.OPS) - 1
        return op

    _REGISTERED["Q1"] = mk("NVFP4_MULCLAMP_ANT", spec_q1)
    _REGISTERED["Q2"] = mk("NVFP4_MAGICRNE_ANT", spec_q2)
    _REGISTERED["Q3A"] = mk("NVFP4_TRUNC1_ANT", spec_q3a)
    _REGISTERED["Q3B"] = mk("NVFP4_COMBINE_ANT", spec_q3b)
    _REGISTERED["Q3B2"] = mk("NVFP4_COMBRNE_ANT", spec_q3b2)
    return _REGISTERED


_NC_CACHE = {}


def build_nc(tloc=TLOC, k=K, out_f=OUT_F, qs=QS, ngrp=NGRP):
    key = (tloc, k, out_f, qs, ngrp)
    if key in _NC_CACHE:
        return _NC_CACHE[key]

    import concourse.bass as bass
    import concourse.mybir as mybir
    import concourse.tile as tile
    from concourse import bacc, masks

    ops = _register_ops()
    dt = mybir.dt

    KT = k // P              # 32 k-tiles
    MT = tloc // P           # 4 local m-tiles
    NT = out_f // P          # 128 n-tiles
    NG = NT // ngrp          # 32 weight groups
    GW = ngrp * P            # 512 out columns per group

    nc = bacc.Bacc("TRN2", target_bir_lowering=False, debug=False,
                   num_devices=N_CORES)

    x_d = nc.dram_tensor("x", [tloc, k], dt.float32, kind="ExternalInput").ap()
    # weights prepacked on host as [g, p, kt, j] so each group is one
    # contiguous 2 MiB block (sequential HBM reads, 16 KiB per partition line)
    wt_d = nc.dram_tensor("wt", [(out_f // (ngrp * P)) * P, k * ngrp],
                          dt.float8e4, kind="ExternalInput").ap()
    b_d = nc.dram_tensor("bias", [P, NT], dt.float32, kind="ExternalInput").ap()
    o_d = nc.dram_tensor("outT", [out_f, tloc], dt.float32, kind="ExternalOutput").ap()

    with tile.TileContext(nc) as tc:
        with (
            tc.tile_pool(name="const", bufs=1) as constp,
            tc.tile_pool(name="xdT", bufs=1) as xdTp,
            tc.tile_pool(name="xin", bufs=16) as xin,
            tc.tile_pool(name="scal", bufs=3) as scal,
            tc.tile_pool(name="v2p", bufs=3) as v2p,
            tc.tile_pool(name="tp", bufs=3) as tp,
            tc.tile_pool(name="q2p", bufs=3) as q2p,
            tc.tile_pool(name="xqp", bufs=6) as xqp,
            tc.tile_pool(name="shp", bufs=3) as shp,
            tc.tile_pool(name="wgp", bufs=3) as wgp,
            tc.tile_pool(name="outp", bufs=3) as outp,
            tc.tile_pool(name="psum", bufs=4, space="PSUM") as psump,
            tc.tile_pool(name="psumT", bufs=4, space="PSUM") as psumTp,
        ):
            # ---- constants ----
            nmask = constp.tile([P, 1], dt.float32, tag="nmask")
            nc.vector._memset_packed(nmask[:], 0xFF800000)
            smask = constp.tile([P, 1], dt.float32, tag="smask")
            nc.vector._memset_packed(smask[:], 0x00400000)
            bias_t = constp.tile([P, NT], dt.float32, tag="bias")
            ident = constp.tile([P, P], dt.bfloat16, tag="ident")
            masks.make_identity(nc, ident[:])

            # ---- resident transposed dequantized activations (fp8, *ASCALE) ----
            xdTa = xdTp.tile([P, KT * tloc], dt.float8e4, tag="xdTa",
                             name="xdTa")
            xdT3 = xdTa[:].rearrange("p (kt t) -> p kt t", t=tloc)

            nq = k // qs             # quant slices per m-tile row
            nblk = qs // BLOCK       # 16-blocks per quant slice
            ntp = qs // P            # transposes per quant slice

            # ---- prefetch all x slices up front (sync ring, q-major order
            # matching consumption) so quant never waits on input DMA ----
            xts = {}
            for q in range(nq):
                for m in range(MT):
                    xsl = xin.tile([P, qs], dt.float32, tag="xsl", name="xsl")
                    nc.sync.dma_start(
                        xsl[:], x_d[m * P:(m + 1) * P, q * qs:(q + 1) * qs])
                    xts[(m, q)] = xsl
            nc.sync.dma_start(bias_t[:], b_d[:, :])

            # ---- prefetch the first weight groups while quant runs ----
            def wload(g):
                wg = wgp.tile([P, KT * GW], dt.float8e4, tag="wg", name="wg")
                nc.sync.dma_start(wg[:], wt_d[g * P:(g + 1) * P, :])
                return wg

            wgs = [wload(0), wload(1), wload(2)]

            def quant_slice(m, q):
                xv = xts[(m, q)][:]
                # scales
                amax = scal.tile([P, nblk], dt.float32, tag="amax")
                nc.vector.tensor_reduce(
                    amax[:], xv.rearrange("p (b s) -> p b s", s=BLOCK),
                    axis=mybir.AxisListType.X, op=mybir.AluOpType.max,
                    apply_absolute_value=True)
                # s8 on DVE: keeps the ACT queue (which gates overlap MMs via
                # the transpose evictions) out of the DVE dependency chain
                s8 = scal.tile([P, nblk], dt.float8e4, tag="s8")
                nc.vector.tensor_scalar(
                    out=s8[:], in0=amax[:], scalar1=1.0 / 6.0, scalar2=None,
                    op0=mybir.AluOpType.mult)
                sh = scal.tile([P, nblk], dt.float32, tag="sh")
                nc.vector.tensor_scalar(
                    out=sh[:], in0=s8[:], scalar1=FP8_MIN, scalar2=0.5,
                    op0=mybir.AluOpType.max, op1=mybir.AluOpType.mult)
                r2 = scal.tile([P, nblk], dt.float32, tag="r2")
                rs = scal.tile([P, nblk], dt.float32, tag="rs")
                nc.vector.reciprocal_approx_accurate(r2[:], sh[:], rs[:])
                # s/2 expanded to bf16 (ACT)
                shx = shp.tile([P, qs], dt.bfloat16, tag="shx")
                nc.scalar.activation(
                    shx[:].rearrange("p (b s) -> p b s", s=BLOCK),
                    sh[:].unsqueeze(2).to_broadcast((P, nblk, BLOCK)),
                    mybir.ActivationFunctionType.Copy, bias=0.0, scale=1.0)
                # v2 = clamp(x * 2/s, +-12)
                v2 = v2p.tile([P, qs], dt.float32, tag="v2")
                nc.vector._custom_dve(
                    ops["Q1"], out=v2[:], in0=xv,
                    in1=r2[:].unsqueeze(2).to_broadcast((P, nblk, BLOCK)),
                    s0=12.0)
                # qh = trunc1(v2 + sign_binade/4)
                qh = tp.tile([P, qs], dt.float32, tag="qh")
                nc.vector._custom_dve(
                    ops["Q3A"], out=qh[:], in0=v2[:],
                    s0=nmask[:, :], s1=smask[:, :], imm2=0.25)
                # q2 = select(qh^2>=16, qh, RNE-to-int(v2)) -> bf16
                # (magic-number RNE fused into the op; no ACT in the chain)
                q2 = q2p.tile([P, qs], dt.bfloat16, tag="q2")
                nc.vector._custom_dve(
                    ops["Q3B2"], out=q2[:], in0=qh[:], in1=v2[:],
                    s0=MAGIC, imm2=16.0)
                # xdeq = q2 * s/2  (bf16 2x mode)
                xq = xqp.tile([P, qs], dt.bfloat16, tag="xq")
                nc.vector.tensor_tensor(
                    out=xq[:], in0=q2[:], in1=shx[:],
                    op=mybir.AluOpType.mult)
                # on-chip transpose into the resident k-major fp8 tile
                # (eviction applies the *ASCALE prescale and the fp8 cast)
                for j in range(ntp):
                    kt = q * ntp + j
                    pst = psumTp.tile([P, P], dt.bfloat16, tag="pst",
                                      name="pst")
                    nc.tensor.transpose(
                        pst[:], xq[:, j * P:(j + 1) * P], ident[:])
                    nc.scalar.activation(
                        xdTa[:, kt * tloc + m * P: kt * tloc + (m + 1) * P],
                        pst[:],
                        mybir.ActivationFunctionType.Copy, bias=0.0,
                        scale=ASCALE)

            def mm_chain(ps, wg, nt):
                # fp8 DoubleRow: one MM contracts a PAIR of k-tiles via 3D APs
                # [p, 2, f]; out += W_k.T @ X_k + W_{k+1}.T @ X_{k+1}
                # k-pairs run HIGH->LOW: the first issued MM needs the last
                # quantized k-tile, so the in-order PE queue blocks exactly
                # once at the quant/matmul boundary instead of stalling the
                # transpose stream with partially-satisfiable chains.
                wg3 = wg[:].rearrange("p (kt j) -> p kt j", j=GW)
                for ki in range(KT - 2, -2, -2):
                    lhsT = wg3[:, ki:ki + 2, nt * P:(nt + 1) * P]
                    rhs = xdT3[:, ki:ki + 2, :]
                    nc.tensor.matmul(
                        ps[:], lhsT, rhs,
                        start=(ki == KT - 2), stop=(ki == 0),
                        perf_mode=mybir.MatmulPerfMode.DoubleRow)

            def evict(ps, n):
                ob = outp.tile([P, tloc], dt.float32, tag="ob", name="ob")
                nc.scalar.activation(
                    ob[:], ps[:], mybir.ActivationFunctionType.Identity,
                    bias=bias_t[:, n:n + 1], scale=OSCALE)
                nc.scalar.dma_start(o_d[n * P:(n + 1) * P, :], ob[:])

            # ---- quant with overlapped group-0 chains: after each quant
            # q-batch, the 4 group-0 output chains consume exactly the
            # freshly-evicted k-tiles (natural k order), so every issued MM's
            # deps are already satisfied and the PE queue never blocks the
            # transpose stream on a partially-satisfiable chain ----
            wg0_3 = wgs[0][:].rearrange("p (kt j) -> p kt j", j=GW)
            ovps = [psump.tile([P, tloc], dt.float32, tag="ps", name="ps")
                    for _ in range(ngrp)]
            kq = KT // nq  # k-tiles completed per quant q-batch
            for q in range(nq):
                for m in range(MT):
                    quant_slice(m, q)
                for nt in range(ngrp):
                    for ki in range(q * kq, (q + 1) * kq, 2):
                        lhsT = wg0_3[:, ki:ki + 2, nt * P:(nt + 1) * P]
                        rhs = xdT3[:, ki:ki + 2, :]
                        nc.tensor.matmul(
                            ovps[nt][:], lhsT, rhs,
                            start=(ki == 0), stop=(ki == KT - 2),
                            perf_mode=mybir.MatmulPerfMode.DoubleRow)
            for nt in range(ngrp):
                evict(ovps[nt], nt)

            # ---- streamed matmul: out^T[n, t] = w_deq @ x_deq^T ----
            for g in range(1, NG):
                wg = wgs[g] if g < 3 else wload(g)
                for nt in range(ngrp):
                    ps = psump.tile([P, tloc], dt.float32, tag="ps", name="ps")
                    mm_chain(ps, wg, nt)
                    evict(ps, g * ngrp + nt)

    nc.compile()
    _NC_CACHE[key] = nc
    return nc


def _prep_weights(weight_q, weight_scale, bias):
    """Host prepack: transposed dequantized bf16 weights (shared by all cores).

    Packed as [g, p, kt, j] (g = 512-out group, p = k-within-tile, kt = k-tile,
    j = out-within-group) so each group is one contiguous 4 MiB DMA with
    32 KiB per SBUF partition line, in the [kt, j] column order the matmul
    slices expect.
    """
    import ml_dtypes
    wq = np.asarray(weight_q, np.float32).reshape(OUT_F, K // BLOCK, BLOCK)
    ws = np.asarray(weight_scale, np.float32)[:, :, None]
    wdeq = (wq * ws).reshape(OUT_F, K)  # exact: <=6 significand bits
    wt = (wdeq.T * np.float32(WSCALE)).astype(ml_dtypes.float8_e4m3)  # [K, OUT_F]
    KT, NG, GW = K // P, OUT_F // (NGRP * P), NGRP * P
    wt_p = np.ascontiguousarray(
        wt.reshape(KT, P, NG, GW).transpose(2, 1, 0, 3)).reshape(NG * P, K * NGRP)
    bias_r = np.ascontiguousarray(
        np.asarray(bias, np.float32).reshape(OUT_F // P, P).T)  # [P, NT]
    return wt_p, bias_r


def kernel(x, weight_q, weight_scale, bias):
    from concourse.bass_utils import run_bass_kernel_spmd

    nc = build_nc()
    x2 = np.ascontiguousarray(np.asarray(x, np.float32).reshape(TOK, K))
    wt, bias_r = _prep_weights(weight_q, weight_scale, bias)
    in_maps = [{"x": x2[c * TLOC:(c + 1) * TLOC], "wt": wt, "bias": bias_r}
               for c in range(N_CORES)]
    res = run_bass_kernel_spmd(nc, in_maps, list(range(N_CORES)))
    out = np.empty((TOK, OUT_F), np.float32)
    for c in range(N_CORES):
        out[c * TLOC:(c + 1) * TLOC, :] = res.results[c]["outT"].T
    return out.reshape(1, TOK, OUT_F)


if __name__ == "__main__":
    rng = np.random.default_rng(0)
    x = rng.normal(size=(1, TOK, K)).astype(np.float32)
    wq = rng.normal(size=(OUT_F, K)).astype(np.float32)
    ws = rng.random(size=(OUT_F, K // BLOCK)).astype(np.float32) + 0.1
    b = rng.normal(size=(OUT_F,)).astype(np.float32)
    out = kernel(x, wq, ws, b)
    print(out.shape, out.dtype)
